# revision 9
# baseline (speedup 1.0000x reference)
"""Trainium2 Bass kernel for nn_AttentionBlock (GroupNorm + qkv conv + head-dim attention + proj + residual).

Sharding: data-parallel over batch B=16 -> 2 batch elements per core on 8 cores.
All heavy matmuls in float32r (full PE rate, ~9e-4 mean rel err); post-softmax
attention path (exp^T, v, proj) in bf16.

GroupNorm is folded into the qkv conv:
  xn = a*x + b2 (per-channel a = gamma*rstd, b2 = beta - mean*a)
  qkv = (W diag(a)) x + (W b2 + b_qkv)
so x is never normalized in memory; weights are rescaled per batch on-chip.
"""
import sys, os
sys.path.insert(0, "/opt/trn_rl_repo")
sys.path.insert(0, "/opt/trn_rl_repo/concourse")
import numpy as np

B, C, H, W = 16, 512, 64, 64
N = H * W            # 4096 spatial
NH = 8               # heads
D = C // NH          # 64 head dim
G = 32               # groups
EPS = 1e-5
NCORES = 8
BPC = B // NCORES    # 2 batches per core

NT = C // 128        # 4 channel tiles
NCHUNK = N // 128    # 32 pixel chunks (qk gen)
NJ = N // 512        # 8 columns blocks of 512

_cache = {}


def _build():
    import concourse.bass as bass
    import concourse.bacc as bacc
    import concourse.tile as tile
    from concourse import mybir
    from concourse.masks import make_identity

    f32 = mybir.dt.float32
    f32r = mybir.dt.float32r
    bf16 = mybir.dt.bfloat16
    AF = mybir.ActivationFunctionType
    ALU = mybir.AluOpType
    AX = mybir.AxisListType

    nc = bacc.Bacc()

    x2 = nc.dram_tensor("x2", [BPC, C, N], f32, kind="ExternalInput")
    wqkT = nc.dram_tensor("wqkT", [C, 3 * C], f32, kind="ExternalInput")   # w_qkv.T  [c, o]
    wpT = nc.dram_tensor("wpT", [C, C], f32, kind="ExternalInput")          # w_proj.T [c, o]
    gamma_pc = nc.dram_tensor("gamma_pc", [128, NT], f32, kind="ExternalInput")
    beta_pc = nc.dram_tensor("beta_pc", [128, NT], f32, kind="ExternalInput")
    bqk_row = nc.dram_tensor("bqk_row", [1, 2 * C], f32, kind="ExternalInput")
    bv_pc = nc.dram_tensor("bv_pc", [128, NT], f32, kind="ExternalInput")
    bp_pc = nc.dram_tensor("bp_pc", [128, NT], f32, kind="ExternalInput")
    out2 = nc.dram_tensor("out2", [BPC, C, N], f32, kind="ExternalOutput")

    with tile.TileContext(nc) as tc:
        with tc.tile_pool(name="consts", bufs=1) as consts, \
             tc.tile_pool(name="wpool", bufs=1) as wpool, \
             tc.tile_pool(name="xpool", bufs=1) as xpool, \
             tc.tile_pool(name="vpool", bufs=1) as vpool, \
             tc.tile_pool(name="work", bufs=2) as work, \
             tc.tile_pool(name="qkcpool", bufs=3) as qkcpool, \
             tc.tile_pool(name="slabpool", bufs=2) as slabpool, \
             tc.tile_pool(name="stagepool", bufs=4) as stagepool, \
             tc.tile_pool(name="ps", bufs=1, space="PSUM") as ps:

            # ---------------- constants / weights (once per core) ----------------
            ident = consts.tile([128, 128], f32, tag="ident")
            make_identity(nc, ident)

            gam = consts.tile([128, NT], f32, tag="gam")
            bet = consts.tile([128, NT], f32, tag="bet")
            bvc = consts.tile([128, NT], f32, tag="bvc")
            bpc_t = consts.tile([128, NT], f32, tag="bpc")
            bqkr = consts.tile([1, 2 * C], f32, tag="bqkr")
            nc.sync.dma_start(out=gam, in_=gamma_pc[:, :])
            nc.sync.dma_start(out=bet, in_=beta_pc[:, :])
            nc.sync.dma_start(out=bvc, in_=bv_pc[:, :])
            nc.sync.dma_start(out=bpc_t, in_=bp_pc[:, :])
            nc.sync.dma_start(out=bqkr, in_=bqk_row[:, :])

            wraw = []
            ws = []
            wp = []
            for t in range(NT):
                wr = wpool.tile([128, 3 * C], f32, tag=f"wraw{t}")
                nc.sync.dma_start(out=wr, in_=wqkT[128 * t:128 * (t + 1), :])
                wraw.append(wr)
                ws_t = wpool.tile([128, 3 * C], f32r, tag=f"ws{t}", name=f"ws{t}")
                ws.append(ws_t)
                w_p = wpool.tile([128, C], bf16, tag=f"wp{t}")
                nc.gpsimd.dma_start(out=w_p, in_=wpT[128 * t:128 * (t + 1), :])
                wp.append(w_p)

            for b in range(BPC):
                # ---------------- load x (fp32 -> f32r cast DMA) ----------------
                xt = []
                for t in range(NT):
                    x_t = xpool.tile([128, N], f32r, tag=f"x{t}")
                    nc.gpsimd.dma_start(out=x_t, in_=x2[b, 128 * t:128 * (t + 1), :])
                    xt.append(x_t)

                # ---------------- group-norm statistics ----------------
                # per-channel bn stats; mv_all[:, s, t]: s=0 mean, s=1 var
                mv_all = work.tile([128, 2, NT], f32, tag="mv_all")
                for t in range(NT):
                    st = work.tile([128, 8, 6], f32, tag="bnstats")
                    xf = xt[t].bitcast(f32)
                    for j in range(8):
                        nc.vector.bn_stats(out=st[:, j, :], in_=xf[:, 512 * j:512 * (j + 1)])
                    nc.vector.bn_aggr(out=mv_all[:, :, t], in_=st)

                # transpose means/vars separately: [128, 4] -> [4, 128]
                tps_m = ps.tile([4, 128], f32, tag="small")
                nc.tensor.transpose(tps_m, mv_all[:, 0, :], ident)
                statTm = work.tile([4, 128], f32, tag="statTm")
                nc.vector.tensor_copy(statTm, tps_m)
                tps_v = ps.tile([4, 128], f32, tag="small")
                nc.tensor.transpose(tps_v, mv_all[:, 1, :], ident)
                statTv = work.tile([4, 128], f32, tag="statTv")
                nc.vector.tensor_copy(statTv, tps_v)

                gsum_m = work.tile([4, 8], f32, tag="gsum_m")
                nc.vector.reduce_sum(out=gsum_m, in_=statTm.rearrange("p (g k) -> p g k", k=16), axis=AX.X)
                gsum_v = work.tile([4, 8], f32, tag="gsum_v")
                nc.vector.reduce_sum(out=gsum_v, in_=statTv.rearrange("p (g k) -> p g k", k=16), axis=AX.X)
                sqmT = work.tile([4, 128], f32, tag="sqmT")
                nc.scalar.activation(out=sqmT, in_=statTm, func=AF.Square)
                sqsum = work.tile([4, 8], f32, tag="sqsum")
                nc.vector.reduce_sum(out=sqsum, in_=sqmT.rearrange("p (g k) -> p g k", k=16), axis=AX.X)

                mean_g = work.tile([4, 8], f32, tag="mean_g")
                nc.scalar.mul(out=mean_g, in_=gsum_m, mul=1.0 / 16.0)
                ex2s = work.tile([4, 8], f32, tag="ex2s")
                nc.vector.tensor_add(ex2s, gsum_v, sqsum)
                msq = work.tile([4, 8], f32, tag="msq")
                nc.scalar.activation(out=msq, in_=mean_g, func=AF.Square)
                var_g = work.tile([4, 8], f32, tag="var_g")
                nc.vector.tensor_scalar(out=var_g, in0=ex2s, scalar1=1.0 / 16.0, scalar2=None, op0=ALU.mult)
                nc.vector.tensor_tensor(var_g, var_g, msq, op=ALU.subtract)
                sdg = work.tile([4, 8], f32, tag="sdg")
                epst = work.tile([4, 1], f32, tag="epst")
                nc.vector.memset(epst, EPS)
                nc.scalar.activation(out=sdg, in_=var_g, func=AF.Sqrt, bias=epst)
                rstd_g = work.tile([4, 8], f32, tag="rstd_g")
                nc.vector.reciprocal(rstd_g, sdg)

                # expand groups (x16) and transpose back to per-channel columns
                import concourse.bass as _bass

                def _rep16(apv):
                    return _bass.AP(tensor=apv.tensor, offset=apv.offset,
                                    ap=[list(apv.ap[0]), list(apv.ap[1]), [0, 16]])

                expand_m = work.tile([4, 128], f32, tag="expand_m")
                expand_r = work.tile([4, 128], f32, tag="expand_r")
                nc.vector.tensor_copy(expand_m.rearrange("p (g k) -> p g k", k=16), _rep16(mean_g[:, :]))
                nc.vector.tensor_copy(expand_r.rearrange("p (g k) -> p g k", k=16), _rep16(rstd_g[:, :]))
                tps2m = ps.tile([128, 4], f32, tag="small")
                nc.tensor.transpose(tps2m, expand_m, ident[0:4, 0:4])
                pc_mean = work.tile([128, NT], f32, tag="pc_mean")
                nc.vector.tensor_copy(pc_mean, tps2m)
                tps2r = ps.tile([128, 4], f32, tag="small")
                nc.tensor.transpose(tps2r, expand_r, ident[0:4, 0:4])
                pc_rstd = work.tile([128, NT], f32, tag="pc_rstd")
                nc.vector.tensor_copy(pc_rstd, tps2r)

                acol = work.tile([128, NT], f32, tag="acol")
                nc.vector.tensor_tensor(acol, pc_rstd, gam, op=ALU.mult)
                b2 = work.tile([128, NT], f32, tag="b2")
                tmpba = work.tile([128, NT], f32, tag="tmpba")
                nc.vector.tensor_tensor(tmpba, pc_mean, acol, op=ALU.mult)
                nc.vector.tensor_tensor(b2, bet, tmpba, op=ALU.subtract)

                # ---------------- scale weights ----------------
                for t in range(NT):
                    nc.vector.tensor_scalar_mul(out=ws[t], in0=wraw[t], scalar1=acol[:, t:t + 1])

                # ---------------- effective biases ----------------
                # qk bias row [1, 1024] = b_qk + (Ws^T b2a)
                brps = ps.tile([1, 1024], f32, tag="big")
                for t in range(NT):
                    for h in range(2):
                        nc.tensor.matmul(brps[:, 512 * h:512 * (h + 1)],
                                         b2[:, t:t + 1], wraw[t][:, 512 * h:512 * (h + 1)],
                                         start=(t == 0), stop=(t == NT - 1))
                qk_bias_row = work.tile([1, 2 * C], f32, tag="qk_bias_row")
                nc.vector.tensor_add(qk_bias_row, brps, bqkr)
                qk_bias_rep = work.tile([128, 2 * C], f32, tag="qk_bias_rep")
                nc.gpsimd.partition_broadcast(qk_bias_rep, qk_bias_row)

                # v bias col [128, NT]
                vbias = work.tile([128, NT], f32, tag="vbias")
                for m in range(NT):
                    vbps = ps.tile([128, 1], f32, tag="small")
                    for t in range(NT):
                        nc.tensor.matmul(vbps, wraw[t][:, 2 * C + 128 * m:2 * C + 128 * (m + 1)],
                                         b2[:, t:t + 1], start=(t == 0), stop=(t == NT - 1))
                    nc.vector.tensor_add(vbias[:, m:m + 1], vbps, bvc[:, m:m + 1])

                # ---------------- qk chunks + score accumulation ----------------
                scps_l = [ps.tile([128, 128], f32, tag=f"scores{p}", name=f"scps{p}") for p in range(4)]
                for ni in range(NCHUNK):
                    cps = ps.tile([128, 1024], f32, tag="big")
                    for h in range(2):
                        for t in range(NT):
                            nc.tensor.matmul(cps[:, 512 * h:512 * (h + 1)],
                                             xt[t][:, 128 * ni:128 * (ni + 1)],
                                             ws[t][:, 512 * h:512 * (h + 1)],
                                             start=(t == 0), stop=(t == NT - 1))
                    qkc = qkcpool.tile([128, 2 * C], f32r, tag="qkc")
                    nc.vector.tensor_add(qkc, cps, qk_bias_rep)
                    for p in range(4):
                        nc.tensor.matmul(scps_l[p],
                                         qkc[:, 128 * p:128 * (p + 1)],
                                         qkc[:, 512 + 128 * p:512 + 128 * (p + 1)],
                                         start=(ni == 0), stop=(ni == NCHUNK - 1),
                                         skip_group_check=True)

                # ---------------- v generation ----------------
                vt = []
                for m in range(NT):
                    v_m = vpool.tile([128, N], bf16, tag=f"v{m}")
                    for nj in range(NJ):
                        vps = ps.tile([128, 512], f32, tag="small")
                        for t in range(NT):
                            nc.tensor.matmul(vps,
                                             ws[t][:, 2 * C + 128 * m:2 * C + 128 * (m + 1)],
                                             xt[t][:, 512 * nj:512 * (nj + 1)],
                                             start=(t == 0), stop=(t == NT - 1))
                        nc.scalar.activation(out=v_m[:, 512 * nj:512 * (nj + 1)], in_=vps,
                                             func=AF.Identity, bias=vbias[:, m:m + 1])
                    vt.append(v_m)

                # ---------------- softmax (per head pair) ----------------
                rden = work.tile([128, 4], f32, tag="rden")
                eTs = []
                for p in range(4):
                    mx = work.tile([128, 1], f32, tag="mx")
                    nc.vector.reduce_max(out=mx[0:64, :], in_=scps_l[p][0:64, 0:64], axis=AX.X)
                    nc.vector.reduce_max(out=mx[64:128, :], in_=scps_l[p][64:128, 64:128], axis=AX.X)
                    negmx = work.tile([128, 1], f32, tag="negmx")
                    nc.scalar.mul(out=negmx, in_=mx, mul=-0.125)
                    e = work.tile([128, 128], f32, tag="exp")
                    nc.vector.memset(e, 0.0)
                    nc.scalar.activation(out=e[0:64, 0:64], in_=scps_l[p][0:64, 0:64],
                                         func=AF.Exp, scale=0.125, bias=negmx[0:64, :])
                    nc.scalar.activation(out=e[64:128, 64:128], in_=scps_l[p][64:128, 64:128],
                                         func=AF.Exp, scale=0.125, bias=negmx[64:128, :])
                    den = work.tile([128, 1], f32, tag="den")
                    nc.vector.reduce_sum(out=den[0:64, :], in_=e[0:64, 0:64], axis=AX.X)
                    nc.vector.reduce_sum(out=den[64:128, :], in_=e[64:128, 64:128], axis=AX.X)
                    nc.vector.reciprocal(rden[:, p:p + 1], den)
                    eps_t = ps.tile([128, 128], f32, tag="small")
                    nc.tensor.transpose(eps_t, e, ident)
                    eT = work.tile([128, 128], bf16, tag=f"eT{p}")
                    nc.vector.tensor_copy(eT, eps_t)
                    eTs.append(eT)

                # ---------------- attention @ v + proj + residual ----------------
                for nj in range(NJ):
                    slabs = []
                    for p in range(4):
                        aps = ps.tile([128, 512], f32, tag="small")
                        nc.tensor.matmul(aps, eTs[p], vt[p][:, 512 * nj:512 * (nj + 1)],
                                         start=True, stop=True)
                        slab = slabpool.tile([128, 512], bf16, tag=f"slab{p}")
                        nc.vector.tensor_scalar_mul(out=slab, in0=aps, scalar1=rden[:, p:p + 1])
                        slabs.append(slab)
                    for m in range(NT):
                        pps = ps.tile([128, 512], f32, tag="big")
                        for p in range(4):
                            nc.tensor.matmul(pps, wp[p][:, 128 * m:128 * (m + 1)], slabs[p],
                                             start=(p == 0), stop=(p == 3))
                        stage = stagepool.tile([128, 512], f32, tag="stage")
                        nc.scalar.activation(out=stage, in_=pps, func=AF.Identity,
                                             bias=bpc_t[:, m:m + 1])
                        nc.vector.tensor_tensor(stage, stage, xt[m].bitcast(f32)[:, 512 * nj:512 * (nj + 1)], op=ALU.add)
                        nc.sync.dma_start(out=out2[b, 128 * m:128 * (m + 1), 512 * nj:512 * (nj + 1)], in_=stage)

    nc.compile()
    return nc


def _get_nc():
    if "nc" not in _cache:
        _cache["nc"] = _build()
    return _cache["nc"]


def kernel(x, gamma, beta, w_qkv, b_qkv, w_proj, b_proj):
    from concourse.bass_utils import run_bass_kernel_spmd

    x = np.asarray(x, dtype=np.float32)
    gamma = np.asarray(gamma, dtype=np.float32)
    beta = np.asarray(beta, dtype=np.float32)
    w_qkv = np.asarray(w_qkv, dtype=np.float32)
    b_qkv = np.asarray(b_qkv, dtype=np.float32)
    w_proj = np.asarray(w_proj, dtype=np.float32)
    b_proj = np.asarray(b_proj, dtype=np.float32)

    nc = _get_nc()

    wqkT = np.ascontiguousarray(w_qkv.T)                       # [512, 1536]
    wpT = np.ascontiguousarray(w_proj.T)                       # [512, 512]
    gamma_pc = np.ascontiguousarray(gamma.reshape(NT, 128).T)  # [128, 4]
    beta_pc = np.ascontiguousarray(beta.reshape(NT, 128).T)
    bqk_row = np.ascontiguousarray(b_qkv[:2 * C].reshape(1, 2 * C))
    bv_pc = np.ascontiguousarray(b_qkv[2 * C:].reshape(NT, 128).T)
    bp_pc = np.ascontiguousarray(b_proj.reshape(NT, 128).T)

    xr = x.reshape(B, C, N)
    in_maps = []
    for i in range(NCORES):
        in_maps.append({
            "x2": np.ascontiguousarray(xr[BPC * i:BPC * (i + 1)]),
            "wqkT": wqkT, "wpT": wpT,
            "gamma_pc": gamma_pc, "beta_pc": beta_pc,
            "bqk_row": bqk_row, "bv_pc": bv_pc, "bp_pc": bp_pc,
        })

    res = run_bass_kernel_spmd(nc, in_maps, core_ids=list(range(NCORES)))
    out = np.empty((B, C, N), dtype=np.float32)
    for i in range(NCORES):
        out[BPC * i:BPC * (i + 1)] = res.results[i]["out2"]
    return out.reshape(B, C, H, W)


# revision 28
# speedup vs baseline: 15.0701x; 15.0701x over previous
"""Trainium2 Bass kernel for nn_AttentionBlock (GroupNorm + qkv conv + head-dim attention + proj + residual).

Sharding: data-parallel over batch B=16 -> 2 batch elements per core on 8 cores.
All heavy matmuls in float32r (full PE rate, ~9e-4 mean rel err); post-softmax
attention path (exp^T, v, proj) in bf16.

GroupNorm is folded into the qkv conv:
  xn = a*x + b2 (per-channel a = gamma*rstd, b2 = beta - mean*a)
  qkv = (W diag(a)) x + (W b2 + b_qkv)
so x is never normalized in memory; weights are rescaled per batch on-chip.
"""
import sys, os
sys.path.insert(0, "/opt/trn_rl_repo")
sys.path.insert(0, "/opt/trn_rl_repo/concourse")
import numpy as np

B, C, H, W = 16, 512, 64, 64
N = H * W            # 4096 spatial
NH = 8               # heads
D = C // NH          # 64 head dim
G = 32               # groups
EPS = 1e-5
NCORES = 8
BPC = B // NCORES    # 2 batches per core

NT = C // 128        # 4 channel tiles
NCHUNK = N // 128    # 32 pixel chunks (qk gen)
NJ = N // 512        # 8 columns blocks of 512

_cache = {}


def _build():
    import concourse.bass as bass
    import concourse.bacc as bacc
    import concourse.tile as tile
    from concourse import mybir
    from concourse.masks import make_identity

    f32 = mybir.dt.float32
    f32r = mybir.dt.float32r
    bf16 = mybir.dt.bfloat16
    AF = mybir.ActivationFunctionType
    ALU = mybir.AluOpType
    AX = mybir.AxisListType

    nc = bacc.Bacc()

    x2 = nc.dram_tensor("x2", [BPC, C, N], f32r, kind="ExternalInput")
    wqkT = nc.dram_tensor("wqkT", [C, 3 * C], f32, kind="ExternalInput")   # w_qkv.T  [c, o]
    wpT = nc.dram_tensor("wpT", [C, C], f32, kind="ExternalInput")          # w_proj.T [c, o]
    gamma_pc = nc.dram_tensor("gamma_pc", [128, NT], f32, kind="ExternalInput")
    beta_pc = nc.dram_tensor("beta_pc", [128, NT], f32, kind="ExternalInput")
    bqk_row = nc.dram_tensor("bqk_row", [1, 2 * C], f32, kind="ExternalInput")
    bv_pc = nc.dram_tensor("bv_pc", [128, NT], f32, kind="ExternalInput")
    bp_row = nc.dram_tensor("bp_row", [1, C], f32, kind="ExternalInput")
    out2 = nc.dram_tensor("out2", [BPC, C, N], f32, kind="ExternalOutput")

    with tile.TileContext(nc) as tc:
        with tc.tile_pool(name="consts", bufs=1) as consts, \
             tc.tile_pool(name="wpool", bufs=1) as wpool, \
             tc.tile_pool(name="xpool", bufs=1) as xpool, \
             tc.tile_pool(name="vpool", bufs=1) as vpool, \
             tc.tile_pool(name="work", bufs=2) as work, \
             tc.tile_pool(name="qkcpool", bufs=4) as qkcpool, \
             tc.tile_pool(name="slabpool", bufs=2) as slabpool, \
             tc.tile_pool(name="stagepool", bufs=4) as stagepool, \
             tc.tile_pool(name="residpool", bufs=3) as residpool, \
             tc.tile_pool(name="ps", bufs=1, space="PSUM") as ps:

            # ---------------- constants / weights (once per core) ----------------
            ident = consts.tile([128, 128], f32, tag="ident")
            make_identity(nc, ident)

            ones_row = consts.tile([1, 512], bf16, tag="ones_row")
            nc.vector.memset(ones_row, 1.0)
            bprow = consts.tile([1, C], bf16, tag="bprow")
            nc.gpsimd.dma_start(out=bprow, in_=bp_row[:, :])
            gam = consts.tile([128, NT], f32, tag="gam")
            bet = consts.tile([128, NT], f32, tag="bet")
            bvc = consts.tile([128, NT], f32, tag="bvc")
            bqkr = consts.tile([1, 2 * C], f32, tag="bqkr")
            nc.sync.dma_start(out=gam, in_=gamma_pc[:, :])
            nc.sync.dma_start(out=bet, in_=beta_pc[:, :])
            nc.sync.dma_start(out=bvc, in_=bv_pc[:, :])
            nc.sync.dma_start(out=bqkr, in_=bqk_row[:, :])

            ws = []
            wp = []
            for t in range(NT):
                ws_t = wpool.tile([128, 3 * C], f32r, tag=f"ws{t}", name=f"ws{t}")
                ws.append(ws_t)
                w_p = wpool.tile([128, C], bf16, tag=f"wp{t}")
                nc.gpsimd.dma_start(out=w_p, in_=wpT[128 * t:128 * (t + 1), :])
                wp.append(w_p)

            for b in range(BPC):
                # ---------------- load x (fp32 -> f32r cast DMA) ----------------
                xt = []
                x_engines = ([nc.sync, nc.gpsimd, nc.scalar] if b == 0
                             else [nc.gpsimd, nc.gpsimd, nc.gpsimd, nc.gpsimd])
                for t in range(NT):
                    x_t = xpool.tile([128, N], f32r, tag=f"x{t}")
                    for q in range(4):
                        x_engines[(t * 4 + q) % len(x_engines)].dma_start(
                            out=x_t[:, 1024 * q:1024 * (q + 1)],
                            in_=x2[b, 128 * t:128 * (t + 1), 1024 * q:1024 * (q + 1)])
                    xt.append(x_t)

                # ---- per-tile stats -> scale/bias -> weight scaling (pipelined) ----
                import concourse.bass as _bass

                def _rep16(apv):
                    return _bass.AP(tensor=apv.tensor, offset=apv.offset,
                                    ap=[list(apv.ap[0]), list(apv.ap[1]), [0, 16]])

                acol = work.tile([128, NT], f32, tag="acol")
                b2 = work.tile([128, NT], f32, tag="b2")
                epst = work.tile([1, 1], f32, tag="epst")
                nc.vector.memset(epst, EPS)
                brps_l = [ps.tile([1, 512], f32, tag="aps", name=f"brps{h}", bufs=2) for h in range(2)]
                vbps_l = [ps.tile([128, 1], f32, tag=f"scores{m}", name=f"vbps{m}") for m in range(NT)]
                for t in range(NT):
                    st = work.tile([128, 8, 6], f32, tag="bnstats")
                    xf = xt[t].bitcast(f32)
                    for j in range(8):
                        nc.vector.bn_stats(out=st[:, j, :], in_=xf[:, 512 * j:512 * (j + 1)])
                    mv = work.tile([128, 2], f32, tag="mv")
                    nc.vector.bn_aggr(out=mv, in_=st)
                    tpm = ps.tile([1, 128], f32, tag="half", name="tpm", bufs=2)
                    nc.tensor.transpose(tpm, mv[:, 0:1], ident)
                    sTm = work.tile([1, 128], f32, tag="sTm")
                    nc.vector.tensor_copy(sTm, tpm)
                    tpv = ps.tile([1, 128], f32, tag="half", name="tpv", bufs=2)
                    nc.tensor.transpose(tpv, mv[:, 1:2], ident)
                    sTv = work.tile([1, 128], f32, tag="sTv")
                    nc.vector.tensor_copy(sTv, tpv)
                    gsm = work.tile([1, 8], f32, tag="gsm")
                    nc.vector.reduce_sum(out=gsm, in_=sTm.rearrange("p (g k) -> p g k", k=16), axis=AX.X)
                    gsv = work.tile([1, 8], f32, tag="gsv")
                    nc.vector.reduce_sum(out=gsv, in_=sTv.rearrange("p (g k) -> p g k", k=16), axis=AX.X)
                    sqm = work.tile([1, 128], f32, tag="sqm")
                    nc.scalar.activation(out=sqm, in_=sTm, func=AF.Square)
                    sqs = work.tile([1, 8], f32, tag="sqs")
                    nc.vector.reduce_sum(out=sqs, in_=sqm.rearrange("p (g k) -> p g k", k=16), axis=AX.X)
                    mean_g = work.tile([1, 8], f32, tag="mean_g")
                    nc.scalar.mul(out=mean_g, in_=gsm, mul=1.0 / 16.0)
                    ex2 = work.tile([1, 8], f32, tag="ex2")
                    nc.vector.tensor_add(ex2, gsv, sqs)
                    msq = work.tile([1, 8], f32, tag="msq")
                    nc.scalar.activation(out=msq, in_=mean_g, func=AF.Square)
                    var_g = work.tile([1, 8], f32, tag="var_g")
                    nc.vector.tensor_scalar(out=var_g, in0=ex2, scalar1=1.0 / 16.0, scalar2=None, op0=ALU.mult)
                    nc.vector.tensor_tensor(var_g, var_g, msq, op=ALU.subtract)
                    sdg = work.tile([1, 8], f32, tag="sdg")
                    nc.scalar.activation(out=sdg, in_=var_g, func=AF.Sqrt, bias=epst)
                    rstd_g = work.tile([1, 8], f32, tag="rstd_g")
                    nc.vector.reciprocal(rstd_g, sdg)
                    em = work.tile([1, 128], f32, tag="em")
                    nc.vector.tensor_copy(em.rearrange("p (g k) -> p g k", k=16), _rep16(mean_g[:, :]))
                    er = work.tile([1, 128], f32, tag="er")
                    nc.vector.tensor_copy(er.rearrange("p (g k) -> p g k", k=16), _rep16(rstd_g[:, :]))
                    tpa = ps.tile([128, 1], f32, tag="half", name="tpa", bufs=2)
                    nc.tensor.transpose(tpa, em, ident[0:1, 0:1])
                    pcm = work.tile([128, 1], f32, tag="pcm")
                    nc.vector.tensor_copy(pcm, tpa)
                    tpb = ps.tile([128, 1], f32, tag="half", name="tpb", bufs=2)
                    nc.tensor.transpose(tpb, er, ident[0:1, 0:1])
                    pcr = work.tile([128, 1], f32, tag="pcr")
                    nc.vector.tensor_copy(pcr, tpb)
                    nc.vector.tensor_tensor(acol[:, t:t + 1], pcr, gam[:, t:t + 1], op=ALU.mult)
                    tmpb = work.tile([128, 1], f32, tag="tmpb")
                    nc.vector.tensor_tensor(tmpb, pcm, acol[:, t:t + 1], op=ALU.mult)
                    nc.vector.tensor_tensor(b2[:, t:t + 1], bet[:, t:t + 1], tmpb, op=ALU.subtract)

                    wtmp = work.tile([128, 3 * C], f32, tag="wtmp")
                    (nc.sync if b == 0 else nc.gpsimd).dma_start(out=wtmp, in_=wqkT[128 * t:128 * (t + 1), :])
                    nc.vector.tensor_scalar_mul(out=ws[t], in0=wtmp, scalar1=acol[:, t:t + 1])
                    for h in range(2):
                        nc.tensor.matmul(brps_l[h], b2[:, t:t + 1], wtmp[:, 512 * h:512 * (h + 1)],
                                         start=(t == 0), stop=(t == NT - 1))
                    for m in range(NT):
                        nc.tensor.matmul(vbps_l[m], wtmp[:, 2 * C + 128 * m:2 * C + 128 * (m + 1)],
                                         b2[:, t:t + 1], start=(t == 0), stop=(t == NT - 1))
                qk_bias_row = work.tile([1, 2 * C], f32, tag="qk_bias_row")
                for h in range(2):
                    nc.vector.tensor_add(qk_bias_row[:, 512 * h:512 * (h + 1)], brps_l[h], bqkr[:, 512 * h:512 * (h + 1)])
                qk_bias_rep = work.tile([128, 2 * C], f32, tag="qk_bias_rep")
                nc.gpsimd.partition_broadcast(qk_bias_rep, qk_bias_row)
                vbias = work.tile([128, NT], f32, tag="vbias")
                for m in range(NT):
                    nc.vector.tensor_add(vbias[:, m:m + 1], vbps_l[m], bvc[:, m:m + 1])

                # ---------------- qk chunks + score accumulation ----------------
                scps_l = [ps.tile([128, 128], f32, tag=f"scores{p}", name=f"scps{p}") for p in range(4)]

                def emit_scores(qkc_prev, nis):
                    for p in range(4):
                        nc.tensor.matmul(scps_l[p],
                                         qkc_prev[:, 128 * p:128 * (p + 1)],
                                         qkc_prev[:, 512 + 128 * p:512 + 128 * (p + 1)],
                                         start=(nis == 0), stop=(nis == NCHUNK - 1),
                                         skip_group_check=True)

                prev_qkc = None
                for ni in range(NCHUNK):
                    qkc = qkcpool.tile([128, 2 * C], f32r, tag="qkc")
                    for h in range(2):
                        cps = ps.tile([128, 512], f32, tag="half", name=f"cps{h}", bufs=2)
                        for t in range(NT):
                            nc.tensor.matmul(cps,
                                             xt[t][:, 128 * ni:128 * (ni + 1)],
                                             ws[t][:, 512 * h:512 * (h + 1)],
                                             start=(t == 0), stop=(t == NT - 1))
                        nc.vector.tensor_add(qkc[:, 512 * h:512 * (h + 1)], cps,
                                             qk_bias_rep[:, 512 * h:512 * (h + 1)])
                    if prev_qkc is not None:
                        emit_scores(prev_qkc, ni - 1)
                    prev_qkc = qkc
                emit_scores(prev_qkc, NCHUNK - 1)

                # ---------------- v generation ----------------
                vt = []
                for m in range(NT):
                    v_m = vpool.tile([128, N], bf16, tag=f"v{m}")
                    for nj in range(NJ):
                        vps = ps.tile([128, 512], f32, tag="half", name="vps", bufs=2)
                        for t in range(NT):
                            nc.tensor.matmul(vps,
                                             ws[t][:, 2 * C + 128 * m:2 * C + 128 * (m + 1)],
                                             xt[t][:, 512 * nj:512 * (nj + 1)],
                                             start=(t == 0), stop=(t == NT - 1))
                        nc.scalar.activation(out=v_m[:, 512 * nj:512 * (nj + 1)], in_=vps,
                                             func=AF.Identity, bias=vbias[:, m:m + 1])
                    vt.append(v_m)

                # ---------------- softmax (per head pair) ----------------
                rden = work.tile([128, 4], f32, tag="rden")
                eTs = []
                for p in range(4):
                    mx = work.tile([128, 1], f32, tag="mx")
                    nc.vector.reduce_max(out=mx[0:64, :], in_=scps_l[p][0:64, 0:64], axis=AX.X)
                    nc.vector.reduce_max(out=mx[64:128, :], in_=scps_l[p][64:128, 64:128], axis=AX.X)
                    negmx = work.tile([128, 1], f32, tag="negmx")
                    nc.scalar.mul(out=negmx, in_=mx, mul=-0.125)
                    e = work.tile([128, 128], f32, tag="exp")
                    nc.vector.memset(e, 0.0)
                    nc.scalar.activation(out=e[0:64, 0:64], in_=scps_l[p][0:64, 0:64],
                                         func=AF.Exp, scale=0.125, bias=negmx[0:64, :])
                    nc.scalar.activation(out=e[64:128, 64:128], in_=scps_l[p][64:128, 64:128],
                                         func=AF.Exp, scale=0.125, bias=negmx[64:128, :])
                    den = work.tile([128, 1], f32, tag="den")
                    nc.vector.reduce_sum(out=den[0:64, :], in_=e[0:64, 0:64], axis=AX.X)
                    nc.vector.reduce_sum(out=den[64:128, :], in_=e[64:128, 64:128], axis=AX.X)
                    nc.vector.reciprocal(rden[:, p:p + 1], den)
                    eps_t = ps.tile([128, 128], f32, tag="aps", name="eps_t", bufs=2)
                    nc.tensor.transpose(eps_t, e, ident)
                    eT = work.tile([128, 128], bf16, tag=f"eT{p}")
                    nc.vector.tensor_copy(eT, eps_t)
                    eTs.append(eT)

                # ---------------- attention @ v + proj + residual ----------------
                def emit_attnv(nj):
                    slabs = []
                    for p in range(4):
                        aps = ps.tile([128, 512], f32, tag="aps", name="aps", bufs=2)
                        nc.tensor.matmul(aps, eTs[p], vt[p][:, 512 * nj:512 * (nj + 1)],
                                         start=True, stop=True)
                        slab = slabpool.tile([128, 512], bf16, tag=f"slab{p}")
                        nc.scalar.activation(out=slab, in_=aps, func=AF.Copy, scale=rden[:, p:p + 1])
                        slabs.append(slab)
                    return slabs

                cur_slabs = emit_attnv(0)
                for nj in range(NJ):
                    next_slabs = emit_attnv(nj + 1) if nj + 1 < NJ else None
                    for m in range(NT):
                        pps = ps.tile([128, 512], f32, tag="half", name="pps", bufs=2)
                        nc.tensor.matmul(pps, bprow[:, 128 * m:128 * (m + 1)], ones_row,
                                         start=True, stop=False)
                        for p in range(4):
                            nc.tensor.matmul(pps, wp[p][:, 128 * m:128 * (m + 1)], cur_slabs[p],
                                             start=False, stop=(p == 3))
                        resid = residpool.tile([128, 512], f32r, tag="resid")
                        resid_eng = [nc.sync, nc.scalar, nc.gpsimd, nc.gpsimd][m]
                        resid_eng.dma_start(out=resid, in_=x2[b, 128 * m:128 * (m + 1), 512 * nj:512 * (nj + 1)])
                        stage = stagepool.tile([128, 512], f32, tag="stage")
                        nc.vector.tensor_tensor(stage, pps, resid.bitcast(f32), op=ALU.add)
                        nc.sync.dma_start(out=out2[b, 128 * m:128 * (m + 1), 512 * nj:512 * (nj + 1)], in_=stage)
                    cur_slabs = next_slabs

    nc.compile()
    return nc


def _get_nc():
    if "nc" not in _cache:
        _cache["nc"] = _build()
    return _cache["nc"]


def kernel(x, gamma, beta, w_qkv, b_qkv, w_proj, b_proj):
    from concourse.bass_utils import run_bass_kernel_spmd

    x = np.asarray(x, dtype=np.float32)
    gamma = np.asarray(gamma, dtype=np.float32)
    beta = np.asarray(beta, dtype=np.float32)
    w_qkv = np.asarray(w_qkv, dtype=np.float32)
    b_qkv = np.asarray(b_qkv, dtype=np.float32)
    w_proj = np.asarray(w_proj, dtype=np.float32)
    b_proj = np.asarray(b_proj, dtype=np.float32)

    nc = _get_nc()

    wqkT = np.ascontiguousarray(w_qkv.T)                       # [512, 1536]
    wpT = np.ascontiguousarray(w_proj.T)                       # [512, 512]
    gamma_pc = np.ascontiguousarray(gamma.reshape(NT, 128).T)  # [128, 4]
    beta_pc = np.ascontiguousarray(beta.reshape(NT, 128).T)
    bqk_row = np.ascontiguousarray(b_qkv[:2 * C].reshape(1, 2 * C))
    bv_pc = np.ascontiguousarray(b_qkv[2 * C:].reshape(NT, 128).T)
    bp_row = np.ascontiguousarray(b_proj.reshape(1, C))

    xr = x.reshape(B, C, N)
    in_maps = []
    for i in range(NCORES):
        in_maps.append({
            "x2": np.ascontiguousarray(xr[BPC * i:BPC * (i + 1)]),
            "wqkT": wqkT, "wpT": wpT,
            "gamma_pc": gamma_pc, "beta_pc": beta_pc,
            "bqk_row": bqk_row, "bv_pc": bv_pc, "bp_row": bp_row,
        })

    res = run_bass_kernel_spmd(nc, in_maps, core_ids=list(range(NCORES)))
    out = np.empty((B, C, N), dtype=np.float32)
    for i in range(NCORES):
        out[BPC * i:BPC * (i + 1)] = res.results[i]["out2"]
    return out.reshape(B, C, H, W)
